# revision 6
# baseline (speedup 1.0000x reference)
"""Supervised-contrastive loss (nn_ConLoss) on 8 Trainium2 NeuronCores.

Strategy (per sharding hint): shard feature rows across the 8 cores
(1024 rows each). Every core holds the full transposed feature matrix in
SBUF — column-permuted so its own 1024 rows sit at columns 0..1023, which
keeps one SPMD program with static APs — computes its [1024, 8192] block
of Z = X X^T / T on the tensor engine (fp32r), does the diagonal-masked
row max / sum-exp / label-mask sums locally, and emits one per-row loss
vector. The host sums the 8 partial vectors and divides by sum(weights).
"""
import numpy as np

TEMPERATURE = 0.1
N, D, C = 8192, 512, 8
R = N // C            # 1024 rows per core
NRB = R // 128        # 8 row blocks of 128
CW = 1024             # elementwise tile width (2 psum banks)
NCC = N // CW         # 8 column chunks

_NC_CACHE = {}


def _build_nc():
    if "nc" in _NC_CACHE:
        return _NC_CACHE["nc"]
    import concourse.tile as tile
    from concourse import bacc, mybir
    from contextlib import ExitStack

    DT = mybir.dt
    ALU = mybir.AluOpType
    ACTF = mybir.ActivationFunctionType

    nc = bacc.Bacc("TRN2", target_bir_lowering=False, debug=False)
    xt_d = nc.dram_tensor("xt", [D, N], DT.float32r, kind="ExternalInput")
    lcol_d = nc.dram_tensor("labcol", [128, N], DT.bfloat16, kind="ExternalInput")
    lrow_d = nc.dram_tensor("labrow", [128, NRB], DT.float32, kind="ExternalInput")
    negw_d = nc.dram_tensor("negw", [128, NRB], DT.float32, kind="ExternalInput")
    icnt_d = nc.dram_tensor("icnt10", [128, NRB], DT.float32, kind="ExternalInput")
    ome_d = nc.dram_tensor("ome", [128, 128], DT.float32, kind="ExternalInput")
    res_d = nc.dram_tensor("res", [NRB, 128], DT.float32, kind="ExternalOutput")

    with tile.TileContext(nc) as tc, ExitStack() as ctx:
        xt_pool = ctx.enter_context(tc.tile_pool(name="xt", bufs=1))
        lab_pool = ctx.enter_context(tc.tile_pool(name="lab", bufs=1))
        small_pool = ctx.enter_context(tc.tile_pool(name="small", bufs=1))
        z_pool = ctx.enter_context(tc.tile_pool(name="z", bufs=1))
        ps_pool = ctx.enter_context(tc.tile_pool(name="ps", bufs=4, space="PSUM"))
        scr_pool = ctx.enter_context(tc.tile_pool(name="scr", bufs=2))
        st_pool = ctx.enter_context(tc.tile_pool(name="st", bufs=3))

        xt_sb = {}
        for cc in range(NCC):
            for k in range(4):
                t = xt_pool.tile([128, CW], DT.float32r, tag=f"xt_{k}_{cc}")
                nc.sync.dma_start(t[:], xt_d[k * 128:(k + 1) * 128, cc * CW:(cc + 1) * CW])
                xt_sb[k, cc] = t
        lcol_sb = []
        for cc in range(NCC):
            t = lab_pool.tile([128, CW], DT.bfloat16, tag=f"lab_{cc}")
            nc.sync.dma_start(t[:], lcol_d[:, cc * CW:(cc + 1) * CW])
            lcol_sb.append(t)
        lrow_sb = small_pool.tile([128, NRB], DT.float32)
        nc.sync.dma_start(lrow_sb[:], lrow_d[:])
        negw_sb = small_pool.tile([128, NRB], DT.float32)
        nc.sync.dma_start(negw_sb[:], negw_d[:])
        icnt_sb = small_pool.tile([128, NRB], DT.float32)
        nc.sync.dma_start(icnt_sb[:], icnt_d[:])
        ome_sb = small_pool.tile([128, 128], DT.float32)
        nc.sync.dma_start(ome_sb[:], ome_d[:])

        for rb in range(NRB):
            rm = st_pool.tile([128, NCC], DT.float32, tag="rm")
            sp = st_pool.tile([128, NCC], DT.float32, tag="sp")
            zp = st_pool.tile([128, NCC], DT.float32, tag="zp")

            z_tiles = []
            for cc in range(NCC):
                ps = ps_pool.tile([128, CW], DT.float32, tag="ps")
                for h in range(2):
                    for k in range(4):
                        nc.tensor.matmul(
                            ps[:, h * 512:(h + 1) * 512],
                            xt_sb[k, 0][:, rb * 128:(rb + 1) * 128],
                            xt_sb[k, cc][:, h * 512:(h + 1) * 512],
                            start=(k == 0), stop=(k == 3))
                if cc == 0:
                    off = rb * 128
                    nc.vector.scalar_tensor_tensor(
                        out=ps[:, off:off + 128], in0=ps[:, off:off + 128],
                        scalar=0.0, in1=ome_sb[:],
                        op0=ALU.bypass, op1=ALU.mult)
                zt = z_pool.tile([128, CW], DT.float32, tag=f"z_{cc}")
                # fused PSUM->SBUF copy + per-tile row max
                nc.vector.tensor_scalar(
                    out=zt[:], in0=ps[:], scalar1=0.0, scalar2=-3.0e38,
                    op0=ALU.add, op1=ALU.max, accum_out=rm[:, cc:cc + 1])
                z_tiles.append(zt)

            mfin = st_pool.tile([128, 1], DT.float32, tag="mfin")
            nc.vector.tensor_reduce(mfin[:], rm[:], axis=mybir.AxisListType.X,
                                    op=ALU.max)
            negm = st_pool.tile([128, 1], DT.float32, tag="negm")
            nc.vector.tensor_scalar_mul(negm[:], mfin[:], -10.0)

            for cc in range(NCC):
                mscr = scr_pool.tile([128, CW], DT.bfloat16, tag="mscr")
                nc.vector.scalar_tensor_tensor(
                    out=mscr[:], in0=lcol_sb[cc][:], scalar=lrow_sb[:, rb:rb + 1],
                    in1=z_tiles[cc][:], op0=ALU.is_equal, op1=ALU.mult,
                    accum_out=zp[:, cc:cc + 1])
                escr = scr_pool.tile([128, CW], DT.bfloat16, tag="escr")
                nc.scalar.activation(
                    out=escr[:], in_=z_tiles[cc][:], func=ACTF.Exp,
                    bias=negm[:], scale=10.0, accum_out=sp[:, cc:cc + 1])

            ssum = st_pool.tile([128, 1], DT.float32, tag="ssum")
            nc.vector.reduce_sum(ssum[:], sp[:], axis=mybir.AxisListType.X)
            lns = st_pool.tile([128, 1], DT.float32, tag="lns")
            nc.scalar.activation(lns[:], ssum[:], ACTF.Ln)
            lse = st_pool.tile([128, 1], DT.float32, tag="lse")
            nc.vector.tensor_sub(lse[:], lns[:], negm[:])
            zsum = st_pool.tile([128, 1], DT.float32, tag="zsum")
            nc.vector.reduce_sum(zsum[:], zp[:], axis=mybir.AxisListType.X)
            tmp = st_pool.tile([128, 1], DT.float32, tag="tmp")
            nc.vector.scalar_tensor_tensor(
                out=tmp[:], in0=zsum[:], scalar=icnt_sb[:, rb:rb + 1], in1=lse[:],
                op0=ALU.mult, op1=ALU.subtract)
            resv = st_pool.tile([128, 1], DT.float32, tag="resv")
            nc.vector.tensor_scalar(
                out=resv[:], in0=tmp[:], scalar1=negw_sb[:, rb:rb + 1],
                scalar2=None, op0=ALU.mult)
            nc.sync.dma_start(res_d[rb, :], resv[:])

    nc.compile()
    _NC_CACHE["nc"] = nc
    return nc


def _reset_device():
    try:
        import ctypes, jax
        jax.devices()
        ctypes.CDLL("/opt/axon/libaxon_pjrt.so").axon_reset()
    except Exception:
        pass


def _make_in_maps(features, labels, weights):
    import ml_dtypes

    f = np.ascontiguousarray(np.asarray(features, dtype=np.float32))
    lab = np.asarray(labels).astype(np.int32)
    w = np.asarray(weights, dtype=np.float32)

    xt = np.ascontiguousarray(f.T)                      # [D, N]
    lab_bf = lab.astype(ml_dtypes.bfloat16)
    ome = (1.0 - np.eye(128)).astype(np.float32)
    hist = np.bincount(lab, minlength=100).astype(np.float64)
    icnt10_full = (10.0 / (hist[lab] - 1.0)).astype(np.float32)   # [N]

    in_maps = []
    for c in range(C):
        sl = slice(c * R, (c + 1) * R)
        perm = np.concatenate([
            np.arange(c * R, (c + 1) * R),
            np.arange(0, c * R),
            np.arange((c + 1) * R, N),
        ])
        in_maps.append({
            "xt": np.ascontiguousarray(xt[:, perm]),
            "labcol": np.ascontiguousarray(
                np.broadcast_to(lab_bf[perm][None, :], (128, N))),
            "labrow": np.ascontiguousarray(
                lab[sl].astype(np.float32).reshape(NRB, 128).T),
            "negw": np.ascontiguousarray(
                (-w[sl]).reshape(NRB, 128).T),
            "icnt10": np.ascontiguousarray(
                icnt10_full[sl].reshape(NRB, 128).T),
            "ome": ome,
        })

    return in_maps


def kernel(features, labels, weights):
    from concourse.bass_utils import run_bass_kernel_spmd

    w = np.asarray(weights, dtype=np.float32)
    nc = _build_nc()
    _reset_device()
    in_maps = _make_in_maps(features, labels, weights)
    out = run_bass_kernel_spmd(nc, in_maps, list(range(C)))
    total = np.float64(0.0)
    for c in range(C):
        total += out.results[c]["res"].astype(np.float64).sum()
    loss = total / np.float64(w.astype(np.float64).sum())
    return np.asarray(loss, dtype=np.float32)
